# revision 28
# baseline (speedup 1.0000x reference)
"""AttentionBlock (GroupNorm + 8-head self-attention + proj + residual) on 8 trn2 cores.

Sharding: data-parallel over batch B=8 -> one batch per NeuronCore. Each core runs
the full block for its batch; no collectives. Host reorders/transposes weights and
gathers per-core outputs.

Per-core dataflow (C=512 channels, L=1024 positions, 8 heads x 64ch):
  GroupNorm   : bn_stats per channel -> group-combine via tiny PE matmuls with an
                indicator matrix -> per-channel scale/shift -> hn (bf16)
  qkv         : hn @ wq/wk (PE, bf16) -> q,k (ch on partitions); v computed
                TRANSPOSED (lhsT=hn) -> vT (s on partitions) so the AV matmul needs
                no transpose. The softmax 1/sqrt(ch) scale is folded into wq.
  attention   : logits computed transposed, wT[s,t] = k^T q (2 heads packed in the
                128-partition dim via tile_position row groups); exp on ScalarE
                (PSUM->SBUF bf16); AV matmul a' = [v;1]^T @ exp(wT) accumulates the
                softmax denominator as a free 65th row; normalize with
                reciprocal + DMA partition-broadcast.
  proj        : a_all @ wproj (PE) + residual (x pre-biased with proj_b).
"""

import math
import os
import sys

import numpy as np

for _p in (
    "/opt/trn_rl_repo",
    "/root/.axon_site",
    "/root/.axon_site/_ro/trn_rl_repo",
    "/root/.axon_site/_ro/pypackages",
):
    if os.path.isdir(_p) and _p not in sys.path:
        sys.path.append(_p)

import ml_dtypes  # noqa: E402

import concourse.bass as bass  # noqa: E402
import concourse.mybir as mybir  # noqa: E402
import concourse.tile as tile  # noqa: E402
from concourse import bacc  # noqa: E402

B, C, HH, WW = 8, 512, 32, 32
L = HH * WW  # 1024
NH, CH = 8, 64  # heads, channels per head
G, GS = 32, 16  # groups, channels per group
EPS = 1e-5
P = 128
NT = C // P  # 4 channel tiles
ST = L // P  # 8 s tiles
NHALF = L // 512  # 2 free-dim halves of 512
F32 = mybir.dt.float32
BF16 = mybir.dt.bfloat16
N_CORES = 8

EW_BUFS = 6


def _emit(tc: tile.TileContext, io: dict):
    nc = tc.nc
    x_d = io["x"].rearrange("(t p) l -> p t l", p=P)
    wqkvT_d = io["wqkvT"].rearrange("(t p) o -> p t o", p=P)
    wprojT_d = io["wprojT"].rearrange("(t p) o -> p t o", p=P)
    gnw_d = io["gn_w"].rearrange("(t p) one -> p t one", p=P)
    gnb_d = io["gn_b"].rearrange("(t p) one -> p t one", p=P)
    bq_d = io["bq"].rearrange("(t p) one -> p t one", p=P)
    bk_d = io["bk"].rearrange("(t p) one -> p t one", p=P)
    bv_d = io["bv_rep"]  # (128, 512) host-replicated
    bproj_d = io["bproj"].rearrange("(t p) one -> p t one", p=P)
    indf_d = io["ind_fwd"].rearrange("(t p) g -> p t g", p=P)  # (128, NT, 32)
    indb_d = io["ind_bwd"].rearrange("g (t p) -> g t p", p=P)  # (32, NT, 128)
    out_d = io["out"].rearrange("(t p) l -> p t l", p=P)
    rcp_scr = nc.dram_tensor("rcp_scratch", [16, 512], F32).ap()

    from contextlib import ExitStack

    with ExitStack() as stack:
        persist = stack.enter_context(tc.tile_pool(name="persist", bufs=1))
        work = stack.enter_context(tc.tile_pool(name="work", bufs=2))
        ew_pool = stack.enter_context(tc.tile_pool(name="ew_pool", bufs=EW_BUFS))
        rep_pool = stack.enter_context(tc.tile_pool(name="rep_pool", bufs=4))
        out_pool = stack.enter_context(tc.tile_pool(name="out_pool", bufs=2))
        ps2 = stack.enter_context(tc.tile_pool(name="ps2", bufs=1, space="PSUM"))
        ps1 = stack.enter_context(tc.tile_pool(name="ps1", bufs=1, space="PSUM"))
        ps_av = stack.enter_context(tc.tile_pool(name="ps_av", bufs=2, space="PSUM"))
        # ---- constant/persistent tiles + loads ----
        xt = persist.tile([P, NT, L], F32, name="xt")
        wqkvT = persist.tile([P, NT, 3 * C], BF16, name="wqkvT")
        wprojT = persist.tile([P, NT, C], BF16, name="wprojT")
        gnw = persist.tile([P, NT, 1], F32, name="gnw")
        gnb = persist.tile([P, NT, 1], F32, name="gnb")
        bq = persist.tile([P, NT, 1], F32, name="bq")
        bk = persist.tile([P, NT, 1], F32, name="bk")
        bv = persist.tile([P, C], F32, name="bv")
        bproj = persist.tile([P, NT, 1], F32, name="bproj")
        indf = persist.tile([P, NT, G], F32, name="indf")
        indb = persist.tile([G, NT, P], F32, name="indb")
        hn = persist.tile([P, NT, L], BF16, name="hn")
        qq = persist.tile([P, NT, L], BF16, name="qq")
        kk_t = persist.tile([P, NT, L], BF16, name="kk_t")
        vT = persist.tile([P, ST, NH * 128], BF16, name="vT")
        a_all = persist.tile([P, NT, L], BF16, name="a_all")
        stats2 = persist.tile([G, 2], F32, name="stats2")

        # tiny tensors first so the GroupNorm chain is never stuck behind the
        # megabyte-sized weight loads in the DMA queues
        nc.sync.dma_start(out=indf[:], in_=indf_d)
        nc.sync.dma_start(out=indb[:], in_=indb_d)
        nc.sync.dma_start(out=gnw[:], in_=gnw_d)
        nc.sync.dma_start(out=gnb[:], in_=gnb_d)
        nc.sync.dma_start(out=bq[:], in_=bq_d)
        nc.sync.dma_start(out=bk[:], in_=bk_d)
        nc.sync.dma_start(out=bv[:], in_=bv_d)
        nc.sync.dma_start(out=bproj[:], in_=bproj_d)
        for t in range(NT):
            for sub in range(2):
                nc.sync.dma_start(
                    out=xt[:, t, sub * 512 : (sub + 1) * 512],
                    in_=x_d[:, t, sub * 512 : (sub + 1) * 512],
                )
        nc.sync.dma_start(out=wqkvT[:], in_=wqkvT_d)
        nc.sync.dma_start(out=wprojT[:], in_=wprojT_d)

        # Head slot layout (128 cols): col 0 = ones (the AV matmul then emits
        # the softmax denominator at PSUM partition 0, where
        # reciprocal_approx_fast works - it breaks at partition offsets on HW),
        # cols 1-63 zero, cols 64-127 = v channels (so the attention rows land
        # at PSUM partitions 64..127, a legal 64-partition engine AP).
        vT_h = vT.rearrange("p s (h x) -> p s h x", x=128)

        # ---- GroupNorm stats ----
        psg_t = ps_av.tile([P, 512], F32, name="psg_t", tag="pav")
        psg = psg_t[0:G, 0:2]
        mm2s = []
        for t in range(NT):
            st6 = work.tile([P, 2, 6], F32, name="st6", tag="st6")
            for sub in range(2):
                nc.vector.bn_stats(
                    out=st6[:, sub, :], in_=xt[:, t, sub * 512 : (sub + 1) * 512]
                )
            mm2 = work.tile([P, 2], F32, name="mm2", tag="mm2", bufs=NT)
            nc.vector.bn_aggr(out=mm2[:], in_=st6[:])  # [mean_c, var_c]
            sq = work.tile([P, 1], F32, name="sq", tag="sq")
            nc.vector.tensor_mul(out=sq[:], in0=mm2[:, 0:1], in1=mm2[:, 0:1])
            nc.vector.tensor_add(out=mm2[:, 1:2], in0=mm2[:, 1:2], in1=sq[:])
            mm2s.append(mm2)
        for t in range(NT):
            nc.tensor.matmul(
                psg[:],
                lhsT=indf[:, t, :],
                rhs=mm2s[t][:],
                start=(t == 0),
                stop=(t == NT - 1),
            )
        # psg = [mean_g, E[x^2]_g]; istd_g = rsqrt(var+eps)
        nc.vector.tensor_copy(out=stats2[:, 0:1], in_=psg[:, 0:1])
        sqg = work.tile([G, 1], F32, name="sqg", tag="sqg")
        nc.vector.tensor_mul(out=sqg[:], in0=stats2[:, 0:1], in1=stats2[:, 0:1])
        varg = work.tile([G, 1], F32, name="varg", tag="varg")
        nc.vector.tensor_sub(out=varg[:], in0=psg[:, 1:2], in1=sqg[:])
        epst = work.tile([G, 1], F32, name="epst", tag="epst")
        nc.vector.memset(epst[:], EPS)
        nc.scalar.activation(
            out=varg[:],
            in_=varg[:],
            func=mybir.ActivationFunctionType.Sqrt,
            bias=epst[:],
        )
        nc.vector.reciprocal(out=stats2[:, 1:2], in_=varg[:])

        # ---- GN apply: hn = x * s_c + t_c ; x += proj_b (pre-bias residual) ----
        for t in range(NT):
            psb_t = ps_av.tile([P, 512], F32, name="psb_t", tag="pav")
            psb = psb_t[0:P, 0:2]
            nc.tensor.matmul(
                psb[:], lhsT=indb[:, t, :], rhs=stats2[:], start=True, stop=True
            )
            sc = work.tile([P, 1], F32, name="sc", tag="sc")
            nc.vector.tensor_mul(out=sc[:], in0=psb[:, 1:2], in1=gnw[:, t, :])
            tc_ = work.tile([P, 1], F32, name="tc_", tag="tc_")
            nc.vector.tensor_mul(out=tc_[:], in0=psb[:, 0:1], in1=sc[:])
            nc.vector.tensor_sub(out=tc_[:], in0=gnb[:, t, :], in1=tc_[:])
            nc.vector.tensor_scalar(
                out=hn[:, t, :],
                in0=xt[:, t, :],
                scalar1=sc[:],
                scalar2=tc_[:],
                op0=mybir.AluOpType.mult,
                op1=mybir.AluOpType.add,
            )
            nc.vector.tensor_scalar_add(
                out=xt[:, t, :], in0=xt[:, t, :], scalar1=bproj[:, t, :]
            )

        # ---- qkv + attention ----
        # PE order hint: vT and the first q/k m-tile first, then one attention
        # head at a time with the remaining qkv m-tiles slotted between heads
        # (the attention window is ScalarE-bound, so PE has slack to absorb
        # them without stalling the exp stream).
        bv3 = bv.rearrange("p (h c) -> p h c", c=CH)
        # vT constant columns on GpSimd - DVE would hoist these into the
        # kernel start and delay bn_stats; GpSimd is otherwise idle
        nc.gpsimd.memset(vT[:], 0.0)
        nc.gpsimd.memset(vT_h[:, :, :, 0:1], 1.0)
        ones64 = persist.tile([1, 64], BF16, name="ones64")
        nc.gpsimd.memset(ones64[:], 1.0)

        def emit_qk(which, dest, bias, ofs, m):
            if m % 2 == 0:
                ps = ps2.tile([P, 2 * L], F32, name=f"ps{which}{m}", tag="ps2")[:, 0:L]
            else:
                ps = ps1.tile([P, L], F32, name=f"ps{which}{m}", tag="ps1")
            for kt in range(NT):
                for n in range(NHALF):
                    nc.tensor.matmul(
                        ps[:, n * 512 : (n + 1) * 512],
                        lhsT=wqkvT[:, kt, ofs + m * P : ofs + (m + 1) * P],
                        rhs=hn[:, kt, n * 512 : (n + 1) * 512],
                        start=(kt == 0),
                        stop=(kt == NT - 1),
                    )
            nc.vector.tensor_scalar_add(
                out=dest[:, m, :], in0=ps[:], scalar1=bias[:, m, :]
            )

        def emit_vt(s):
            if s % 2 == 0:
                psv = ps2.tile([P, 2 * L], F32, name=f"psvT{s}", tag="ps2")[:, 0:L]
            else:
                psv = ps1.tile([P, L], F32, name=f"psvT{s}", tag="ps1")
            for kt in range(NT):
                nc.tensor.matmul(
                    psv[:, 0:512],
                    lhsT=hn[:, kt, s * P : (s + 1) * P],
                    rhs=wqkvT[:, kt, 2 * C : 3 * C],
                    start=(kt == 0),
                    stop=(kt == NT - 1),
                )
            nc.vector.tensor_tensor(
                out=vT_h[:, s, :, 64:128],
                in0=psv[:, 0:512].rearrange("p (h c) -> p h c", c=CH),
                in1=bv3,
                op=mybir.AluOpType.add,
            )

        def emit_head(h, flush_pending=None):
            pr, part = h // 2, (h % 2) * 64
            pv = []
            for half in range(NHALF):
                pv_t = ps_av.tile([P, 512], F32, name=f"pav{h}{half}", tag="pav")
                pv.append(pv_t)
            for j in range(ST):
                psL = ps_big.tile([P, L], F32, name=f"pg{h}{j}", tag="psL")
                for n in range(NHALF):
                    nc.tensor.matmul(
                        psL[:, n * 512 : (n + 1) * 512],
                        lhsT=kk_t[part : part + 64, pr, j * P : (j + 1) * P],
                        rhs=qq[part : part + 64, pr, n * 512 : (n + 1) * 512],
                        start=True,
                        stop=True,
                        tile_position=(part, 0),
                    )
                ew = ew_pool.tile([P, L], BF16, name=f"ew{h}{j}", tag="ew")
                nc.scalar.activation(
                    out=ew[:], in_=psL[:], func=mybir.ActivationFunctionType.Exp
                )
                for half in range(NHALF):
                    nc.tensor.matmul(
                        pv[half][:],
                        lhsT=vT[:, j, h * 128 : h * 128 + 128],
                        rhs=ew[:, half * 512 : (half + 1) * 512],
                        start=(j == 0),
                        stop=(j == ST - 1),
                    )
                if j == 1 and flush_pending is not None:
                    flush_pending()
            return pv

        def emit_drains(h, pv):
            pr, part = h // 2, (h % 2) * 64
            # normalize: a = a' / sumexp (denominator at partition 0). The
            # staging copy frees the accumulator bank fast; the reciprocal row
            # is broadcast across partitions with a K=1 ones matmul (on-chip,
            # low latency - this chain is the proj tail's critical path).
            for half in range(NHALF):
                psv_ = pv[half]
                stg = rep_pool.tile([P, 512], F32, name="stg", tag="stg", bufs=4)
                nc.vector.tensor_copy(out=stg[:], in_=psv_[:])
                rcpf = work.tile([1, 512], F32, name="rcpf", tag="rcpf", bufs=4)
                nc.vector.reciprocal_approx_fast(out=rcpf[:], in_=stg[0:1, :])
                rcpb = work.tile([1, 512], BF16, name="rcpb", tag="rcpb", bufs=4)
                nc.vector.tensor_copy(out=rcpb[:], in_=rcpf[:])
                rep_t = ps_av.tile([P, 512], F32, name="rep_t", tag="pav")
                nc.tensor.matmul(
                    rep_t[64:128, :],
                    lhsT=ones64[:],
                    rhs=rcpb[:],
                    start=True,
                    stop=True,
                    tile_position=(0, 64),
                )
                nc.vector.tensor_tensor(
                    out=a_all[part : part + 64, pr, half * 512 : (half + 1) * 512],
                    in0=stg[64:128, :],
                    in1=rep_t[64:128, :],
                    op=mybir.AluOpType.mult,
                )

        emit_qk("q", qq, bq, 0, 0)
        emit_qk("k", kk_t, bk, C, 0)
        for m in range(1, NT):
            emit_qk("q", qq, bq, 0, m)
            emit_qk("k", kk_t, bk, C, m)
        for s in range(ST):
            emit_vt(s)
        pending = [None]

        def _flush():
            if pending[0] is not None:
                ph, ppv = pending[0]
                pending[0] = None
                emit_drains(ph, ppv)

        for h in range(NH):
            pv = emit_head(h, flush_pending=_flush)
            pending[0] = (h, pv)

        # ---- proj + residual ----
        # Wave A (m=0,1) accumulates k-tiles 0-2 while the last head drains on
        # DVE; the k=3 matmuls (which need the last head's a_all slices) and
        # wave B follow.
        def emit_proj_mms(ps, m, kts):
            for n in range(NHALF):
                for kt in kts:
                    nc.tensor.matmul(
                        ps[:, n * 512 : (n + 1) * 512],
                        lhsT=wprojT[:, kt, m * P : (m + 1) * P],
                        rhs=a_all[:, kt, n * 512 : (n + 1) * 512],
                        start=(kt == 0),
                        stop=(kt == NT - 1),
                    )

        def emit_proj_tail(ps, m):
            ot = out_pool.tile([P, L], F32, name="ot", tag="ot")
            for half in range(NHALF):
                sl = slice(half * 512, (half + 1) * 512)
                nc.vector.tensor_tensor(
                    out=ot[:, sl], in0=ps[:, sl], in1=xt[:, m, sl],
                    op=mybir.AluOpType.add,
                )
                nc.sync.dma_start(out=out_d[:, m, sl], in_=ot[:, sl])

        ps_a = {}
        for m in (0, 1, 2):
            ps_a[m] = ps_big.tile([P, L], F32, name=f"pspj{m}", tag="psL")
            emit_proj_mms(ps_a[m], m, (0, 1, 2))
        _flush()
        for m in (0, 1, 2):
            emit_proj_mms(ps_a[m], m, (3,))
            emit_proj_tail(ps_a[m], m)
        ps3 = ps_big.tile([P, L], F32, name="pspj3", tag="psL")
        emit_proj_mms(ps3, 3, (0, 1, 2, 3))
        emit_proj_tail(ps3, 3)


# revision 29
# speedup vs baseline: 1.1509x; 1.1509x over previous
"""AttentionBlock (GroupNorm + 8-head self-attention + proj + residual) on 8 trn2 cores.

Sharding: data-parallel over batch B=8 -> one batch per NeuronCore. Each core runs
the full block for its batch; no collectives. Host reorders/transposes weights and
gathers per-core outputs.

Per-core dataflow (C=512 channels, L=1024 positions, 8 heads x 64ch):
  GroupNorm   : bn_stats per channel -> group-combine via tiny PE matmuls with an
                indicator matrix -> per-channel scale/shift -> hn (bf16)
  qkv         : hn @ wq/wk (PE, bf16) -> q,k (ch on partitions); v computed
                TRANSPOSED (lhsT=hn) -> vT (s on partitions) so the AV matmul needs
                no transpose. The softmax 1/sqrt(ch) scale is folded into wq.
  attention   : logits computed transposed, wT[s,t] = k^T q (2 heads packed in the
                128-partition dim via tile_position row groups); exp on ScalarE
                (PSUM->SBUF bf16); AV matmul a' = [v;1]^T @ exp(wT) accumulates the
                softmax denominator as a free 65th row; normalize with
                reciprocal + DMA partition-broadcast.
  proj        : a_all @ wproj (PE) + residual (x pre-biased with proj_b).
"""

import math
import os
import sys

import numpy as np

for _p in (
    "/opt/trn_rl_repo",
    "/root/.axon_site",
    "/root/.axon_site/_ro/trn_rl_repo",
    "/root/.axon_site/_ro/pypackages",
):
    if os.path.isdir(_p) and _p not in sys.path:
        sys.path.append(_p)

import ml_dtypes  # noqa: E402

import concourse.bass as bass  # noqa: E402
import concourse.mybir as mybir  # noqa: E402
import concourse.tile as tile  # noqa: E402
from concourse import bacc  # noqa: E402

B, C, HH, WW = 8, 512, 32, 32
L = HH * WW  # 1024
NH, CH = 8, 64  # heads, channels per head
G, GS = 32, 16  # groups, channels per group
EPS = 1e-5
P = 128
NT = C // P  # 4 channel tiles
ST = L // P  # 8 s tiles
NHALF = L // 512  # 2 free-dim halves of 512
F32 = mybir.dt.float32
BF16 = mybir.dt.bfloat16
N_CORES = 8

EW_BUFS = 6


def _emit(tc: tile.TileContext, io: dict):
    nc = tc.nc
    x_d = io["x"].rearrange("(t p) l -> p t l", p=P)
    wqkvT_d = io["wqkvT"].rearrange("(t p) o -> p t o", p=P)
    wprojT_d = io["wprojT"].rearrange("(t p) o -> p t o", p=P)
    gnw_d = io["gn_w"].rearrange("(t p) one -> p t one", p=P)
    gnb_d = io["gn_b"].rearrange("(t p) one -> p t one", p=P)
    bq_d = io["bq"].rearrange("(t p) one -> p t one", p=P)
    bk_d = io["bk"].rearrange("(t p) one -> p t one", p=P)
    bv_d = io["bv_rep"]  # (128, 512) host-replicated
    bproj_d = io["bproj"].rearrange("(t p) one -> p t one", p=P)
    indf_d = io["ind_fwd"].rearrange("(t p) g -> p t g", p=P)  # (128, NT, 32)
    indb_d = io["ind_bwd"].rearrange("g (t p) -> g t p", p=P)  # (32, NT, 128)
    out_d = io["out"].rearrange("(t p) l -> p t l", p=P)
    rcp_scr = nc.dram_tensor("rcp_scratch", [16, 512], F32).ap()

    from contextlib import ExitStack

    with ExitStack() as stack:
        persist = stack.enter_context(tc.tile_pool(name="persist", bufs=1))
        work = stack.enter_context(tc.tile_pool(name="work", bufs=2))
        ew_pool = stack.enter_context(tc.tile_pool(name="ew_pool", bufs=EW_BUFS))
        rep_pool = stack.enter_context(tc.tile_pool(name="rep_pool", bufs=4))
        out_pool = stack.enter_context(tc.tile_pool(name="out_pool", bufs=2))
        ps_big = stack.enter_context(tc.tile_pool(name="ps_big", bufs=3, space="PSUM"))
        ps_av = stack.enter_context(tc.tile_pool(name="ps_av", bufs=2, space="PSUM"))
        # ---- constant/persistent tiles + loads ----
        xt = persist.tile([P, NT, L], F32, name="xt")
        wqkvT = persist.tile([P, NT, 3 * C], BF16, name="wqkvT")
        wprojT = persist.tile([P, NT, C], BF16, name="wprojT")
        gnw = persist.tile([P, NT, 1], F32, name="gnw")
        gnb = persist.tile([P, NT, 1], F32, name="gnb")
        bq = persist.tile([P, NT, 1], F32, name="bq")
        bk = persist.tile([P, NT, 1], F32, name="bk")
        bv = persist.tile([P, C], F32, name="bv")
        bproj = persist.tile([P, NT, 1], F32, name="bproj")
        indf = persist.tile([P, NT, G], F32, name="indf")
        indb = persist.tile([G, NT, P], F32, name="indb")
        hn = persist.tile([P, NT, L], BF16, name="hn")
        qq = persist.tile([P, NT, L], BF16, name="qq")
        kk_t = persist.tile([P, NT, L], BF16, name="kk_t")
        vT = persist.tile([P, ST, NH * 128], BF16, name="vT")
        a_all = persist.tile([P, NT, L], BF16, name="a_all")
        stats2 = persist.tile([G, 2], F32, name="stats2")

        # tiny tensors first so the GroupNorm chain is never stuck behind the
        # megabyte-sized weight loads in the DMA queues
        nc.sync.dma_start(out=indf[:], in_=indf_d)
        nc.sync.dma_start(out=indb[:], in_=indb_d)
        nc.sync.dma_start(out=gnw[:], in_=gnw_d)
        nc.sync.dma_start(out=gnb[:], in_=gnb_d)
        nc.sync.dma_start(out=bq[:], in_=bq_d)
        nc.sync.dma_start(out=bk[:], in_=bk_d)
        nc.sync.dma_start(out=bv[:], in_=bv_d)
        nc.sync.dma_start(out=bproj[:], in_=bproj_d)
        for t in range(NT):
            for sub in range(2):
                nc.sync.dma_start(
                    out=xt[:, t, sub * 512 : (sub + 1) * 512],
                    in_=x_d[:, t, sub * 512 : (sub + 1) * 512],
                )
        nc.sync.dma_start(out=wqkvT[:], in_=wqkvT_d)
        nc.sync.dma_start(out=wprojT[:], in_=wprojT_d)

        # Head slot layout (128 cols): col 0 = ones (the AV matmul then emits
        # the softmax denominator at PSUM partition 0, where
        # reciprocal_approx_fast works - it breaks at partition offsets on HW),
        # cols 1-63 zero, cols 64-127 = v channels (so the attention rows land
        # at PSUM partitions 64..127, a legal 64-partition engine AP).
        vT_h = vT.rearrange("p s (h x) -> p s h x", x=128)

        # ---- GroupNorm stats ----
        psg_t = ps_av.tile([P, 512], F32, name="psg_t", tag="pav")
        psg = psg_t[0:G, 0:2]
        mm2s = []
        for t in range(NT):
            st6 = work.tile([P, 2, 6], F32, name="st6", tag="st6")
            for sub in range(2):
                nc.vector.bn_stats(
                    out=st6[:, sub, :], in_=xt[:, t, sub * 512 : (sub + 1) * 512]
                )
            mm2 = work.tile([P, 2], F32, name="mm2", tag="mm2", bufs=NT)
            nc.vector.bn_aggr(out=mm2[:], in_=st6[:])  # [mean_c, var_c]
            sq = work.tile([P, 1], F32, name="sq", tag="sq")
            nc.vector.tensor_mul(out=sq[:], in0=mm2[:, 0:1], in1=mm2[:, 0:1])
            nc.vector.tensor_add(out=mm2[:, 1:2], in0=mm2[:, 1:2], in1=sq[:])
            mm2s.append(mm2)
        for t in range(NT):
            nc.tensor.matmul(
                psg[:],
                lhsT=indf[:, t, :],
                rhs=mm2s[t][:],
                start=(t == 0),
                stop=(t == NT - 1),
            )
        # psg = [mean_g, E[x^2]_g]; istd_g = rsqrt(var+eps)
        nc.vector.tensor_copy(out=stats2[:, 0:1], in_=psg[:, 0:1])
        sqg = work.tile([G, 1], F32, name="sqg", tag="sqg")
        nc.vector.tensor_mul(out=sqg[:], in0=stats2[:, 0:1], in1=stats2[:, 0:1])
        varg = work.tile([G, 1], F32, name="varg", tag="varg")
        nc.vector.tensor_sub(out=varg[:], in0=psg[:, 1:2], in1=sqg[:])
        epst = work.tile([G, 1], F32, name="epst", tag="epst")
        nc.vector.memset(epst[:], EPS)
        nc.scalar.activation(
            out=varg[:],
            in_=varg[:],
            func=mybir.ActivationFunctionType.Sqrt,
            bias=epst[:],
        )
        nc.vector.reciprocal(out=stats2[:, 1:2], in_=varg[:])

        # ---- GN apply: hn = x * s_c + t_c ; x += proj_b (pre-bias residual) ----
        for t in range(NT):
            psb_t = ps_av.tile([P, 512], F32, name="psb_t", tag="pav")
            psb = psb_t[0:P, 0:2]
            nc.tensor.matmul(
                psb[:], lhsT=indb[:, t, :], rhs=stats2[:], start=True, stop=True
            )
            sc = work.tile([P, 1], F32, name="sc", tag="sc")
            nc.vector.tensor_mul(out=sc[:], in0=psb[:, 1:2], in1=gnw[:, t, :])
            tc_ = work.tile([P, 1], F32, name="tc_", tag="tc_")
            nc.vector.tensor_mul(out=tc_[:], in0=psb[:, 0:1], in1=sc[:])
            nc.vector.tensor_sub(out=tc_[:], in0=gnb[:, t, :], in1=tc_[:])
            nc.vector.tensor_scalar(
                out=hn[:, t, :],
                in0=xt[:, t, :],
                scalar1=sc[:],
                scalar2=tc_[:],
                op0=mybir.AluOpType.mult,
                op1=mybir.AluOpType.add,
            )
            nc.vector.tensor_scalar_add(
                out=xt[:, t, :], in0=xt[:, t, :], scalar1=bproj[:, t, :]
            )

        # ---- qkv + attention ----
        # PE order hint: vT and the first q/k m-tile first, then one attention
        # head at a time with the remaining qkv m-tiles slotted between heads
        # (the attention window is ScalarE-bound, so PE has slack to absorb
        # them without stalling the exp stream).
        bv3 = bv.rearrange("p (h c) -> p h c", c=CH)
        # vT constant columns on GpSimd - DVE would hoist these into the
        # kernel start and delay bn_stats; GpSimd is otherwise idle
        nc.gpsimd.memset(vT[:], 0.0)
        nc.gpsimd.memset(vT_h[:, :, :, 0:1], 1.0)
        ones64 = persist.tile([1, 64], BF16, name="ones64")
        nc.gpsimd.memset(ones64[:], 1.0)

        def emit_qk(which, dest, bias, ofs, m):
            ps = ps_big.tile([P, L], F32, name=f"ps{which}{m}", tag="psL")
            for kt in range(NT):
                for n in range(NHALF):
                    nc.tensor.matmul(
                        ps[:, n * 512 : (n + 1) * 512],
                        lhsT=wqkvT[:, kt, ofs + m * P : ofs + (m + 1) * P],
                        rhs=hn[:, kt, n * 512 : (n + 1) * 512],
                        start=(kt == 0),
                        stop=(kt == NT - 1),
                    )
            nc.vector.tensor_scalar_add(
                out=dest[:, m, :], in0=ps[:], scalar1=bias[:, m, :]
            )

        def emit_vt(s):
            psv = ps_big.tile([P, L], F32, name=f"psvT{s}", tag="psL")
            for kt in range(NT):
                nc.tensor.matmul(
                    psv[:, 0:512],
                    lhsT=hn[:, kt, s * P : (s + 1) * P],
                    rhs=wqkvT[:, kt, 2 * C : 3 * C],
                    start=(kt == 0),
                    stop=(kt == NT - 1),
                )
            nc.vector.tensor_tensor(
                out=vT_h[:, s, :, 64:128],
                in0=psv[:, 0:512].rearrange("p (h c) -> p h c", c=CH),
                in1=bv3,
                op=mybir.AluOpType.add,
            )

        def emit_head(h, flush_pending=None):
            pr, part = h // 2, (h % 2) * 64
            pv = []
            for half in range(NHALF):
                pv_t = ps_av.tile([P, 512], F32, name=f"pav{h}{half}", tag="pav")
                pv.append(pv_t)
            for j in range(ST):
                psL = ps_big.tile([P, L], F32, name=f"pg{h}{j}", tag="psL")
                for n in range(NHALF):
                    nc.tensor.matmul(
                        psL[:, n * 512 : (n + 1) * 512],
                        lhsT=kk_t[part : part + 64, pr, j * P : (j + 1) * P],
                        rhs=qq[part : part + 64, pr, n * 512 : (n + 1) * 512],
                        start=True,
                        stop=True,
                        tile_position=(part, 0),
                    )
                ew = ew_pool.tile([P, L], BF16, name=f"ew{h}{j}", tag="ew")
                nc.scalar.activation(
                    out=ew[:], in_=psL[:], func=mybir.ActivationFunctionType.Exp
                )
                for half in range(NHALF):
                    nc.tensor.matmul(
                        pv[half][:],
                        lhsT=vT[:, j, h * 128 : h * 128 + 128],
                        rhs=ew[:, half * 512 : (half + 1) * 512],
                        start=(j == 0),
                        stop=(j == ST - 1),
                    )
                if j == 1 and flush_pending is not None:
                    flush_pending()
            return pv

        def emit_drains(h, pv):
            pr, part = h // 2, (h % 2) * 64
            # normalize: a = a' / sumexp (denominator at partition 0). The
            # staging copy frees the accumulator bank fast; the reciprocal row
            # is broadcast across partitions with a K=1 ones matmul (on-chip,
            # low latency - this chain is the proj tail's critical path).
            for half in range(NHALF):
                psv_ = pv[half]
                stg = rep_pool.tile([P, 512], F32, name="stg", tag="stg", bufs=4)
                nc.vector.tensor_copy(out=stg[:], in_=psv_[:])
                rcpf = work.tile([1, 512], F32, name="rcpf", tag="rcpf", bufs=4)
                nc.vector.reciprocal_approx_fast(out=rcpf[:], in_=stg[0:1, :])
                rcpb = work.tile([1, 512], BF16, name="rcpb", tag="rcpb", bufs=4)
                nc.vector.tensor_copy(out=rcpb[:], in_=rcpf[:])
                rep_t = ps_av.tile([P, 512], F32, name="rep_t", tag="pav")
                nc.tensor.matmul(
                    rep_t[64:128, :],
                    lhsT=ones64[:],
                    rhs=rcpb[:],
                    start=True,
                    stop=True,
                    tile_position=(0, 64),
                )
                nc.vector.tensor_tensor(
                    out=a_all[part : part + 64, pr, half * 512 : (half + 1) * 512],
                    in0=stg[64:128, :],
                    in1=rep_t[64:128, :],
                    op=mybir.AluOpType.mult,
                )

        emit_qk("q", qq, bq, 0, 0)
        emit_qk("k", kk_t, bk, C, 0)
        for m in range(1, NT):
            emit_qk("q", qq, bq, 0, m)
            emit_qk("k", kk_t, bk, C, m)
        for s in range(ST):
            emit_vt(s)
        pending = [None]

        def _flush():
            if pending[0] is not None:
                ph, ppv = pending[0]
                pending[0] = None
                emit_drains(ph, ppv)

        for h in range(NH):
            pv = emit_head(h, flush_pending=_flush)
            pending[0] = (h, pv)

        # ---- proj + residual ----
        # Wave A (m=0,1) accumulates k-tiles 0-2 while the last head drains on
        # DVE; the k=3 matmuls (which need the last head's a_all slices) and
        # wave B follow.
        def emit_proj_mms(ps, m, kts):
            for n in range(NHALF):
                for kt in kts:
                    nc.tensor.matmul(
                        ps[:, n * 512 : (n + 1) * 512],
                        lhsT=wprojT[:, kt, m * P : (m + 1) * P],
                        rhs=a_all[:, kt, n * 512 : (n + 1) * 512],
                        start=(kt == 0),
                        stop=(kt == NT - 1),
                    )

        def emit_proj_tail(ps, m):
            ot = out_pool.tile([P, L], F32, name="ot", tag="ot")
            for half in range(NHALF):
                sl = slice(half * 512, (half + 1) * 512)
                nc.vector.tensor_tensor(
                    out=ot[:, sl], in0=ps[:, sl], in1=xt[:, m, sl],
                    op=mybir.AluOpType.add,
                )
                nc.sync.dma_start(out=out_d[:, m, sl], in_=ot[:, sl])

        ps_a = {}
        for m in (0, 1, 2):
            ps_a[m] = ps_big.tile([P, L], F32, name=f"pspj{m}", tag="psL")
            emit_proj_mms(ps_a[m], m, (0, 1, 2))
        _flush()
        for m in (0, 1, 2):
            emit_proj_mms(ps_a[m], m, (3,))
            emit_proj_tail(ps_a[m], m)
        ps3 = ps_big.tile([P, L], F32, name="pspj3", tag="psL")
        emit_proj_mms(ps3, 3, (0, 1, 2, 3))
        emit_proj_tail(ps3, 3)


# revision 31
# speedup vs baseline: 1.3586x; 1.1805x over previous
"""AttentionBlock (GroupNorm + 8-head self-attention + proj + residual) on 8 trn2 cores.

Sharding: data-parallel over batch B=8 -> one batch per NeuronCore. Each core runs
the full block for its batch; no collectives. Host reorders/transposes weights and
gathers per-core outputs.

Per-core dataflow (C=512 channels, L=1024 positions, 8 heads x 64ch):
  GroupNorm   : bn_stats per channel -> group-combine via tiny PE matmuls with an
                indicator matrix -> per-channel scale/shift -> hn (bf16)
  qkv         : hn @ wq/wk (PE, bf16) -> q,k (ch on partitions); v computed
                TRANSPOSED (lhsT=hn) -> vT (s on partitions) so the AV matmul needs
                no transpose. The softmax 1/sqrt(ch) scale is folded into wq.
  attention   : logits computed transposed, wT[s,t] = k^T q (2 heads packed in the
                128-partition dim via tile_position row groups); exp on ScalarE
                (PSUM->SBUF bf16); AV matmul a' = [v;1]^T @ exp(wT) accumulates the
                softmax denominator as a free 65th row; normalize with
                reciprocal + DMA partition-broadcast.
  proj        : a_all @ wproj (PE) + residual (x pre-biased with proj_b).
"""

import math
import os
import sys

import numpy as np

for _p in (
    "/opt/trn_rl_repo",
    "/root/.axon_site",
    "/root/.axon_site/_ro/trn_rl_repo",
    "/root/.axon_site/_ro/pypackages",
):
    if os.path.isdir(_p) and _p not in sys.path:
        sys.path.append(_p)

import ml_dtypes  # noqa: E402

import concourse.bass as bass  # noqa: E402
import concourse.mybir as mybir  # noqa: E402
import concourse.tile as tile  # noqa: E402
from concourse import bacc  # noqa: E402

B, C, HH, WW = 8, 512, 32, 32
L = HH * WW  # 1024
NH, CH = 8, 64  # heads, channels per head
G, GS = 32, 16  # groups, channels per group
EPS = 1e-5
P = 128
NT = C // P  # 4 channel tiles
ST = L // P  # 8 s tiles
NHALF = L // 512  # 2 free-dim halves of 512
F32 = mybir.dt.float32
BF16 = mybir.dt.bfloat16
N_CORES = 8

EW_BUFS = 6


def _emit(tc: tile.TileContext, io: dict):
    nc = tc.nc
    x_d = io["x"].rearrange("(t p) l -> p t l", p=P)
    wqkvT_d = io["wqkvT"].rearrange("(t p) o -> p t o", p=P)
    wprojT_d = io["wprojT"].rearrange("(t p) o -> p t o", p=P)
    gnw_d = io["gn_w"].rearrange("(t p) one -> p t one", p=P)
    gnb_d = io["gn_b"].rearrange("(t p) one -> p t one", p=P)
    bq_d = io["bq"].rearrange("(t p) one -> p t one", p=P)
    bk_d = io["bk"].rearrange("(t p) one -> p t one", p=P)
    bv_d = io["bv_rep"]  # (128, 512) host-replicated
    bproj_d = io["bproj"].rearrange("(t p) one -> p t one", p=P)
    indf_d = io["ind_fwd"].rearrange("(t p) g -> p t g", p=P)  # (128, NT, 32)
    indb_d = io["ind_bwd"].rearrange("g (t p) -> g t p", p=P)  # (32, NT, 128)
    out_d = io["out"].rearrange("(t p) l -> p t l", p=P)
    rcp_scr = nc.dram_tensor("rcp_scratch", [16, 512], F32).ap()

    from contextlib import ExitStack

    with ExitStack() as stack:
        persist = stack.enter_context(tc.tile_pool(name="persist", bufs=1))
        work = stack.enter_context(tc.tile_pool(name="work", bufs=2))
        ew_pool = stack.enter_context(tc.tile_pool(name="ew_pool", bufs=EW_BUFS))
        rep_pool = stack.enter_context(tc.tile_pool(name="rep_pool", bufs=4))
        out_pool = stack.enter_context(tc.tile_pool(name="out_pool", bufs=2))
        ps_big = stack.enter_context(tc.tile_pool(name="ps_big", bufs=3, space="PSUM"))
        ps_av = stack.enter_context(tc.tile_pool(name="ps_av", bufs=2, space="PSUM"))
        # ---- constant/persistent tiles + loads ----
        xt = persist.tile([P, NT, L], F32, name="xt")
        wqkvT = persist.tile([P, NT, 3 * C], BF16, name="wqkvT")
        wprojT = persist.tile([P, NT, C], BF16, name="wprojT")
        gnw = persist.tile([P, NT, 1], F32, name="gnw")
        gnb = persist.tile([P, NT, 1], F32, name="gnb")
        bq = persist.tile([P, NT, 1], F32, name="bq")
        bk = persist.tile([P, NT, 1], F32, name="bk")
        bv = persist.tile([P, C], F32, name="bv")
        bproj = persist.tile([P, NT, 1], F32, name="bproj")
        indf = persist.tile([P, NT, G], F32, name="indf")
        indb = persist.tile([G, NT, P], F32, name="indb")
        hn = persist.tile([P, NT, L], BF16, name="hn")
        qq = persist.tile([P, NT, L], BF16, name="qq")
        kk_t = persist.tile([P, NT, L], BF16, name="kk_t")
        vT = persist.tile([P, ST, NH * 128], BF16, name="vT")
        a_all = persist.tile([P, NT, L], BF16, name="a_all")
        stats2 = persist.tile([G, 2], F32, name="stats2")

        # tiny tensors first so the GroupNorm chain is never stuck behind the
        # megabyte-sized weight loads in the DMA queues
        nc.sync.dma_start(out=indf[:], in_=indf_d)
        nc.sync.dma_start(out=indb[:], in_=indb_d)
        nc.sync.dma_start(out=gnw[:], in_=gnw_d)
        nc.sync.dma_start(out=gnb[:], in_=gnb_d)
        nc.sync.dma_start(out=bq[:], in_=bq_d)
        nc.sync.dma_start(out=bk[:], in_=bk_d)
        nc.sync.dma_start(out=bv[:], in_=bv_d)
        nc.sync.dma_start(out=bproj[:], in_=bproj_d)
        for t in range(NT):
            for sub in range(2):
                nc.sync.dma_start(
                    out=xt[:, t, sub * 512 : (sub + 1) * 512],
                    in_=x_d[:, t, sub * 512 : (sub + 1) * 512],
                )
        nc.sync.dma_start(out=wqkvT[:], in_=wqkvT_d)
        nc.sync.dma_start(out=wprojT[:], in_=wprojT_d)

        # Head slot layout (128 cols): col 0 = ones (the AV matmul then emits
        # the softmax denominator at PSUM partition 0, where
        # reciprocal_approx_fast works - it breaks at partition offsets on HW),
        # cols 1-63 zero, cols 64-127 = v channels (so the attention rows land
        # at PSUM partitions 64..127, a legal 64-partition engine AP).
        vT_h = vT.rearrange("p s (h x) -> p s h x", x=128)

        # ---- GroupNorm stats ----
        psg_t = ps_av.tile([P, 512], F32, name="psg_t", tag="pav")
        psg = psg_t[0:G, 0:2]
        mm2s = []
        for t in range(NT):
            st6 = work.tile([P, 2, 6], F32, name="st6", tag="st6")
            for sub in range(2):
                nc.vector.bn_stats(
                    out=st6[:, sub, :], in_=xt[:, t, sub * 512 : (sub + 1) * 512]
                )
            mm2 = work.tile([P, 2], F32, name="mm2", tag="mm2", bufs=NT)
            nc.vector.bn_aggr(out=mm2[:], in_=st6[:])  # [mean_c, var_c]
            sq = work.tile([P, 1], F32, name="sq", tag="sq")
            nc.vector.tensor_mul(out=sq[:], in0=mm2[:, 0:1], in1=mm2[:, 0:1])
            nc.vector.tensor_add(out=mm2[:, 1:2], in0=mm2[:, 1:2], in1=sq[:])
            mm2s.append(mm2)
        for t in range(NT):
            nc.tensor.matmul(
                psg[:],
                lhsT=indf[:, t, :],
                rhs=mm2s[t][:],
                start=(t == 0),
                stop=(t == NT - 1),
            )
        # psg = [mean_g, E[x^2]_g]; istd_g = rsqrt(var+eps)
        nc.vector.tensor_copy(out=stats2[:, 0:1], in_=psg[:, 0:1])
        sqg = work.tile([G, 1], F32, name="sqg", tag="sqg")
        nc.vector.tensor_mul(out=sqg[:], in0=stats2[:, 0:1], in1=stats2[:, 0:1])
        varg = work.tile([G, 1], F32, name="varg", tag="varg")
        nc.vector.tensor_sub(out=varg[:], in0=psg[:, 1:2], in1=sqg[:])
        epst = work.tile([G, 1], F32, name="epst", tag="epst")
        nc.vector.memset(epst[:], EPS)
        nc.scalar.activation(
            out=varg[:],
            in_=varg[:],
            func=mybir.ActivationFunctionType.Sqrt,
            bias=epst[:],
        )
        nc.vector.reciprocal(out=stats2[:, 1:2], in_=varg[:])

        # ---- GN apply: hn = x * s_c + t_c ; x += proj_b (pre-bias residual) ----
        for t in range(NT):
            psb_t = ps_av.tile([P, 512], F32, name="psb_t", tag="pav")
            psb = psb_t[0:P, 0:2]
            nc.tensor.matmul(
                psb[:], lhsT=indb[:, t, :], rhs=stats2[:], start=True, stop=True
            )
            sc = work.tile([P, 1], F32, name="sc", tag="sc", bufs=4)
            nc.vector.tensor_mul(out=sc[:], in0=psb[:, 1:2], in1=gnw[:, t, :])
            tc_ = work.tile([P, 1], F32, name="tc_", tag="tc_", bufs=4)
            nc.vector.tensor_mul(out=tc_[:], in0=psb[:, 0:1], in1=sc[:])
            nc.vector.tensor_sub(out=tc_[:], in0=gnb[:, t, :], in1=tc_[:])
            if t % 2 == 0:
                # ScalarE is idle here; Copy(x*scale+bias) with per-partition
                # scale/bias vectors is exactly the GN affine
                nc.scalar.activation(
                    out=hn[:, t, :],
                    in_=xt[:, t, :],
                    func=mybir.ActivationFunctionType.Identity,
                    bias=tc_[:],
                    scale=sc[:],
                )
            else:
                nc.vector.tensor_scalar(
                    out=hn[:, t, :],
                    in0=xt[:, t, :],
                    scalar1=sc[:],
                    scalar2=tc_[:],
                    op0=mybir.AluOpType.mult,
                    op1=mybir.AluOpType.add,
                )
        for t in range(NT):
            # residual pre-bias, off the hn critical chain (needed only at proj)
            nc.vector.tensor_scalar_add(
                out=xt[:, t, :], in0=xt[:, t, :], scalar1=bproj[:, t, :]
            )

        # ---- qkv + attention ----
        # PE order hint: vT and the first q/k m-tile first, then one attention
        # head at a time with the remaining qkv m-tiles slotted between heads
        # (the attention window is ScalarE-bound, so PE has slack to absorb
        # them without stalling the exp stream).
        bv3 = bv.rearrange("p (h c) -> p h c", c=CH)
        # vT constant columns on GpSimd - DVE would hoist these into the
        # kernel start and delay bn_stats; GpSimd is otherwise idle
        nc.gpsimd.memset(vT[:], 0.0)
        nc.gpsimd.memset(vT_h[:, :, :, 0:1], 1.0)
        ones64 = persist.tile([1, 64], BF16, name="ones64")
        nc.gpsimd.memset(ones64[:], 1.0)

        def emit_qk(which, dest, bias, ofs, m):
            ps = ps_big.tile([P, L], F32, name=f"ps{which}{m}", tag="psL")
            for kt in range(NT):
                for n in range(NHALF):
                    nc.tensor.matmul(
                        ps[:, n * 512 : (n + 1) * 512],
                        lhsT=wqkvT[:, kt, ofs + m * P : ofs + (m + 1) * P],
                        rhs=hn[:, kt, n * 512 : (n + 1) * 512],
                        start=(kt == 0),
                        stop=(kt == NT - 1),
                    )
            nc.vector.tensor_scalar_add(
                out=dest[:, m, :], in0=ps[:], scalar1=bias[:, m, :]
            )

        def emit_vt(s):
            psv = ps_big.tile([P, L], F32, name=f"psvT{s}", tag="psL")
            for kt in range(NT):
                nc.tensor.matmul(
                    psv[:, 0:512],
                    lhsT=hn[:, kt, s * P : (s + 1) * P],
                    rhs=wqkvT[:, kt, 2 * C : 3 * C],
                    start=(kt == 0),
                    stop=(kt == NT - 1),
                )
            nc.vector.tensor_tensor(
                out=vT_h[:, s, :, 64:128],
                in0=psv[:, 0:512].rearrange("p (h c) -> p h c", c=CH),
                in1=bv3,
                op=mybir.AluOpType.add,
            )

        def emit_head(h, flush_pending=None):
            pr, part = h // 2, (h % 2) * 64
            pv = []
            for half in range(NHALF):
                pv_t = ps_av.tile([P, 512], F32, name=f"pav{h}{half}", tag="pav")
                pv.append(pv_t)
            for j in range(ST):
                psL = ps_big.tile([P, L], F32, name=f"pg{h}{j}", tag="psL")
                for n in range(NHALF):
                    nc.tensor.matmul(
                        psL[:, n * 512 : (n + 1) * 512],
                        lhsT=kk_t[part : part + 64, pr, j * P : (j + 1) * P],
                        rhs=qq[part : part + 64, pr, n * 512 : (n + 1) * 512],
                        start=True,
                        stop=True,
                        tile_position=(part, 0),
                    )
                ew = ew_pool.tile([P, L], BF16, name=f"ew{h}{j}", tag="ew")
                nc.scalar.activation(
                    out=ew[:], in_=psL[:], func=mybir.ActivationFunctionType.Exp
                )
                for half in range(NHALF):
                    nc.tensor.matmul(
                        pv[half][:],
                        lhsT=vT[:, j, h * 128 : h * 128 + 128],
                        rhs=ew[:, half * 512 : (half + 1) * 512],
                        start=(j == 0),
                        stop=(j == ST - 1),
                    )
                if j == 1 and flush_pending is not None:
                    flush_pending()
            return pv

        def emit_drains(h, pv):
            pr, part = h // 2, (h % 2) * 64
            use_act = h == NH - 1  # ScalarE is idle once the exp stream ends
            # normalize: a = a' / sumexp (denominator at partition 0). The
            # staging copy frees the accumulator bank fast; the reciprocal row
            # is broadcast across partitions with a K=1 ones matmul (on-chip,
            # low latency - this chain is the proj tail's critical path).
            for half in range(NHALF):
                psv_ = pv[half]
                stg = rep_pool.tile([P, 512], F32, name="stg", tag="stg", bufs=4)
                if use_act:
                    nc.scalar.activation(
                        out=stg[:], in_=psv_[:],
                        func=mybir.ActivationFunctionType.Copy,
                    )
                else:
                    nc.vector.tensor_copy(out=stg[:], in_=psv_[:])
                rcpf = work.tile([1, 512], F32, name="rcpf", tag="rcpf", bufs=4)
                nc.vector.reciprocal_approx_fast(out=rcpf[:], in_=stg[0:1, :])
                rcpb = work.tile([1, 512], BF16, name="rcpb", tag="rcpb", bufs=4)
                nc.vector.tensor_copy(out=rcpb[:], in_=rcpf[:])
                rep_t = ps_av.tile([P, 512], F32, name="rep_t", tag="pav")
                nc.tensor.matmul(
                    rep_t[64:128, :],
                    lhsT=ones64[:],
                    rhs=rcpb[:],
                    start=True,
                    stop=True,
                    tile_position=(0, 64),
                )
                nc.vector.tensor_tensor(
                    out=a_all[part : part + 64, pr, half * 512 : (half + 1) * 512],
                    in0=stg[64:128, :],
                    in1=rep_t[64:128, :],
                    op=mybir.AluOpType.mult,
                )

        emit_qk("q", qq, bq, 0, 0)
        emit_qk("k", kk_t, bk, C, 0)
        for m in range(1, NT):
            emit_qk("q", qq, bq, 0, m)
            emit_qk("k", kk_t, bk, C, m)
        for s in range(ST):
            emit_vt(s)
        pending = [None]

        def _flush():
            if pending[0] is not None:
                ph, ppv = pending[0]
                pending[0] = None
                emit_drains(ph, ppv)

        for h in range(NH):
            pv = emit_head(h, flush_pending=_flush)
            pending[0] = (h, pv)

        # ---- proj + residual ----
        # Wave A (m=0,1) accumulates k-tiles 0-2 while the last head drains on
        # DVE; the k=3 matmuls (which need the last head's a_all slices) and
        # wave B follow.
        def emit_proj_mms(ps, m, kts):
            for n in range(NHALF):
                for kt in kts:
                    nc.tensor.matmul(
                        ps[:, n * 512 : (n + 1) * 512],
                        lhsT=wprojT[:, kt, m * P : (m + 1) * P],
                        rhs=a_all[:, kt, n * 512 : (n + 1) * 512],
                        start=(kt == 0),
                        stop=(kt == NT - 1),
                    )

        def emit_proj_tail(ps, m):
            ot = out_pool.tile([P, L], F32, name="ot", tag="ot")
            for half in range(NHALF):
                sl = slice(half * 512, (half + 1) * 512)
                nc.vector.tensor_tensor(
                    out=ot[:, sl], in0=ps[:, sl], in1=xt[:, m, sl],
                    op=mybir.AluOpType.add,
                )
                nc.sync.dma_start(out=out_d[:, m, sl], in_=ot[:, sl])

        ps_a = {}
        for m in (0, 1, 2):
            ps_a[m] = ps_big.tile([P, L], F32, name=f"pspj{m}", tag="psL")
            emit_proj_mms(ps_a[m], m, (0, 1, 2))
        _flush()
        for m in (0, 1, 2):
            emit_proj_mms(ps_a[m], m, (3,))
            emit_proj_tail(ps_a[m], m)
        ps3 = ps_big.tile([P, L], F32, name="pspj3", tag="psL")
        emit_proj_mms(ps3, 3, (0, 1, 2, 3))
        emit_proj_tail(ps3, 3)
